# revision 10
# baseline (speedup 1.0000x reference)
"""Trainium2 Bass kernel for nn_LyotFilter: out = x @ w_norm(weight_).

Strategy (data-parallel over 8 NeuronCores):
  - Host: compute the tiny [200, 64] normalized filter matrix in float32
    (mimicking the reference's f32 arithmetic), and reshape each core's
    row-shard of x into a transposed, contiguous [200, 62500] layout so the
    contraction dim (200 features) lands on SBUF partitions with fully
    contiguous per-partition DMA.
  - Device (per core): stream xT chunks HBM->SBUF, TensorE matmul
    (K split 128+72, PSUM fp32 accumulate), DVE copy PSUM->SBUF, stream
    out.T [64, 62500] back to HBM.  Memory-bound: ~66 MB HBM traffic/core.
  - Host: concatenate the 8 [64, 62500] results and transpose to [500000, 64].
"""

import functools

import numpy as np

N_CORES = 8
ROWS = 500000
RPC = ROWS // N_CORES  # 62500 rows per core
IN_DIM = 200
OUT_DIM = 64
K1 = 128               # first contraction chunk (partition limit)
K2 = IN_DIM - K1       # 72
F_CHUNK = 6250         # columns of xT per DMA chunk (10 chunks/core)
INNER = 500            # matmul moving free dim (<=512 for fp32 PSUM bank)


def _w_norm(weight_: np.ndarray) -> np.ndarray:
    """[200, 64] filter matrix, float32 arithmetic mimicking the reference."""
    n = np.arange(220)
    skip = ((n >= 103) & (n <= 107)) | ((n >= 149) & (n <= 162)) | (n == 219)
    kept = n[~skip]
    bands = (400.0 + (2500.0 - 400.0) * kept / 220.0).astype(np.float32)
    num = np.float32(2.0 * np.pi * (-0.01))
    denom = weight_.astype(np.float32)[:, None] * (bands[None, :] * np.float32(1e-6))
    phase = (num / denom).astype(np.float32)
    w = np.float32(0.5) - np.float32(0.5) * np.cos(phase)
    wt = w.T  # [200, 64]
    wn = np.maximum(wt, np.float32(0.0)) / wt.sum(axis=0, dtype=np.float32)
    return np.ascontiguousarray(wn.astype(np.float32))


@functools.cache
def _build():
    import concourse.bass as bass
    import concourse.tile as tile
    from concourse import bacc, mybir

    f32 = mybir.dt.float32
    # float32r: same fp32 bits, but the PE streams it at 1 cycle/row for
    # moving dim >=256 (vs 4 for plain fp32).  ~1.5e-4 matmul error
    # (TF32-class) vs the fp32 reference -- verified end-to-end.
    f32r = mybir.dt.float32r
    nc = bacc.Bacc(
        "TRN2", target_bir_lowering=False, debug=False, num_devices=N_CORES
    )
    xt = nc.declare_dram_parameter("xt", [IN_DIM, RPC], f32r, isOutput=False)
    wn = nc.declare_dram_parameter("wn", [IN_DIM, OUT_DIM], f32r, isOutput=False)
    out = nc.declare_dram_parameter("out_t", [OUT_DIM, RPC], f32, isOutput=True)

    with tile.TileContext(nc) as tc:
        with (
            tc.tile_pool(name="w", bufs=1) as wp,
            tc.tile_pool(name="xt1", bufs=2) as p1,
            tc.tile_pool(name="xt2", bufs=2) as p2,
            tc.tile_pool(name="outp", bufs=2) as po,
            tc.tile_pool(name="ps", bufs=6, space=bass.MemorySpace.PSUM) as pp,
        ):
            w1 = wp.tile([K1, OUT_DIM], f32r, tag="w1")
            w2 = wp.tile([K2, OUT_DIM], f32r, tag="w2")
            nc.sync.dma_start(w1[:], wn[0:K1, :])
            nc.scalar.dma_start(w2[:], wn[K1:IN_DIM, :])

            for ci, f0 in enumerate(range(0, RPC, F_CHUNK)):
                fs = min(F_CHUNK, RPC - f0)
                t1 = p1.tile([K1, F_CHUNK], f32r, tag="xt1")
                t2 = p2.tile([K2, F_CHUNK], f32r, tag="xt2")
                # split input streams across the two HWDGE rings,
                # alternating per chunk to balance bytes between rings
                eng_a = nc.sync if ci % 2 == 0 else nc.scalar
                eng_b = nc.scalar if ci % 2 == 0 else nc.sync
                eng_a.dma_start(t1[:, :fs], xt[0:K1, f0 : f0 + fs])
                eng_b.dma_start(t2[:, :fs], xt[K1:IN_DIM, f0 : f0 + fs])

                ot = po.tile([OUT_DIM, F_CHUNK], f32, tag="out")
                j = 0
                while j < fs:
                    nn = min(INNER, fs - j)
                    ps = pp.tile([OUT_DIM, INNER], f32, tag="ps")
                    nc.tensor.matmul(
                        ps[:, :nn], w1[:], t1[:, j : j + nn], start=True, stop=False
                    )
                    nc.tensor.matmul(
                        ps[:, :nn], w2[:], t2[:, j : j + nn], start=False, stop=True
                    )
                    nc.vector.tensor_copy(ot[:, j : j + nn], ps[:, :nn])
                    j += nn
                # outputs ride the SWDGE ring so they don't head-of-line
                # block the next chunk's input loads; two half-stores so
                # the first half overlaps the second half's compute
                h = fs // 2
                nc.gpsimd.dma_start(out[:, f0 : f0 + h], ot[:, :h])
                nc.gpsimd.dma_start(out[:, f0 + h : f0 + fs], ot[:, h:fs])
    nc.compile()
    return nc


def _run(in_maps, trace=False, **kw):
    from concourse.bass_utils import run_bass_kernel_spmd

    nc = _build()
    return run_bass_kernel_spmd(nc, in_maps, list(range(N_CORES)), trace=trace, **kw)


def _make_in_maps(x: np.ndarray, weight_: np.ndarray):
    wn = _w_norm(weight_)
    in_maps = []
    for i in range(N_CORES):
        xti = np.ascontiguousarray(x[i * RPC : (i + 1) * RPC, :].T.astype(np.float32))
        in_maps.append({"xt": xti, "wn": wn})
    return in_maps


def kernel(x: np.ndarray, weight_: np.ndarray) -> np.ndarray:
    x = np.asarray(x)
    weight_ = np.asarray(weight_)
    res = _run(_make_in_maps(x, weight_)).results
    out_t = np.concatenate([res[i]["out_t"] for i in range(N_CORES)], axis=1)
    return np.ascontiguousarray(out_t.T).astype(np.float32)


# revision 11
# speedup vs baseline: 1.0199x; 1.0199x over previous
"""Trainium2 Bass kernel for nn_LyotFilter: out = x @ w_norm(weight_).

Strategy (data-parallel over 8 NeuronCores):
  - Host: compute the tiny [200, 64] normalized filter matrix in float32
    (mimicking the reference's f32 arithmetic), and reshape each core's
    row-shard of x into a transposed, contiguous [200, 62500] layout so the
    contraction dim (200 features) lands on SBUF partitions with fully
    contiguous per-partition DMA.
  - Device (per core): stream xT chunks HBM->SBUF, TensorE matmul
    (K split 128+72, PSUM fp32 accumulate), DVE copy PSUM->SBUF, stream
    out.T [64, 62500] back to HBM.  Memory-bound: ~66 MB HBM traffic/core.
  - Host: concatenate the 8 [64, 62500] results and transpose to [500000, 64].
"""

import functools

import numpy as np

N_CORES = 8
ROWS = 500000
RPC = ROWS // N_CORES  # 62500 rows per core
IN_DIM = 200
OUT_DIM = 64
K1 = 128               # first contraction chunk (partition limit)
K2 = IN_DIM - K1       # 72
F_CHUNK = 5000         # columns of xT per DMA chunk (12.5 chunks/core)
INNER = 500            # matmul moving free dim (<=512 for fp32 PSUM bank)


def _w_norm(weight_: np.ndarray) -> np.ndarray:
    """[200, 64] filter matrix, float32 arithmetic mimicking the reference."""
    n = np.arange(220)
    skip = ((n >= 103) & (n <= 107)) | ((n >= 149) & (n <= 162)) | (n == 219)
    kept = n[~skip]
    bands = (400.0 + (2500.0 - 400.0) * kept / 220.0).astype(np.float32)
    num = np.float32(2.0 * np.pi * (-0.01))
    denom = weight_.astype(np.float32)[:, None] * (bands[None, :] * np.float32(1e-6))
    phase = (num / denom).astype(np.float32)
    w = np.float32(0.5) - np.float32(0.5) * np.cos(phase)
    wt = w.T  # [200, 64]
    wn = np.maximum(wt, np.float32(0.0)) / wt.sum(axis=0, dtype=np.float32)
    return np.ascontiguousarray(wn.astype(np.float32))


@functools.cache
def _build():
    import concourse.bass as bass
    import concourse.tile as tile
    from concourse import bacc, mybir

    f32 = mybir.dt.float32
    # float32r: same fp32 bits, but the PE streams it at 1 cycle/row for
    # moving dim >=256 (vs 4 for plain fp32).  ~1.5e-4 matmul error
    # (TF32-class) vs the fp32 reference -- verified end-to-end.
    f32r = mybir.dt.float32r
    nc = bacc.Bacc(
        "TRN2", target_bir_lowering=False, debug=False, num_devices=N_CORES
    )
    xt = nc.declare_dram_parameter("xt", [IN_DIM, RPC], f32r, isOutput=False)
    wn = nc.declare_dram_parameter("wn", [IN_DIM, OUT_DIM], f32r, isOutput=False)
    out = nc.declare_dram_parameter("out_t", [OUT_DIM, RPC], f32, isOutput=True)

    with tile.TileContext(nc) as tc:
        with (
            tc.tile_pool(name="w", bufs=1) as wp,
            tc.tile_pool(name="xt1", bufs=3) as p1,
            tc.tile_pool(name="xt2", bufs=3) as p2,
            tc.tile_pool(name="outp", bufs=2) as po,
            tc.tile_pool(name="ps", bufs=6, space=bass.MemorySpace.PSUM) as pp,
        ):
            w1 = wp.tile([K1, OUT_DIM], f32r, tag="w1")
            w2 = wp.tile([K2, OUT_DIM], f32r, tag="w2")
            nc.sync.dma_start(w1[:], wn[0:K1, :])
            nc.scalar.dma_start(w2[:], wn[K1:IN_DIM, :])

            for ci, f0 in enumerate(range(0, RPC, F_CHUNK)):
                fs = min(F_CHUNK, RPC - f0)
                t1 = p1.tile([K1, F_CHUNK], f32r, tag="xt1")
                t2 = p2.tile([K2, F_CHUNK], f32r, tag="xt2")
                # split input streams across the two HWDGE rings,
                # alternating per chunk to balance bytes between rings
                eng_a = nc.sync if ci % 2 == 0 else nc.scalar
                eng_b = nc.scalar if ci % 2 == 0 else nc.sync
                eng_a.dma_start(t1[:, :fs], xt[0:K1, f0 : f0 + fs])
                eng_b.dma_start(t2[:, :fs], xt[K1:IN_DIM, f0 : f0 + fs])

                ot = po.tile([OUT_DIM, F_CHUNK], f32, tag="out")
                j = 0
                while j < fs:
                    nn = min(INNER, fs - j)
                    ps = pp.tile([OUT_DIM, INNER], f32, tag="ps")
                    nc.tensor.matmul(
                        ps[:, :nn], w1[:], t1[:, j : j + nn], start=True, stop=False
                    )
                    nc.tensor.matmul(
                        ps[:, :nn], w2[:], t2[:, j : j + nn], start=False, stop=True
                    )
                    nc.vector.tensor_copy(ot[:, j : j + nn], ps[:, :nn])
                    j += nn
                # outputs ride the SWDGE ring so they don't head-of-line
                # block the next chunk's input loads
                nc.gpsimd.dma_start(out[:, f0 : f0 + fs], ot[:, :fs])
    nc.compile()
    return nc


def _run(in_maps, trace=False, **kw):
    from concourse.bass_utils import run_bass_kernel_spmd

    nc = _build()
    return run_bass_kernel_spmd(nc, in_maps, list(range(N_CORES)), trace=trace, **kw)


def _make_in_maps(x: np.ndarray, weight_: np.ndarray):
    wn = _w_norm(weight_)
    in_maps = []
    for i in range(N_CORES):
        xti = np.ascontiguousarray(x[i * RPC : (i + 1) * RPC, :].T.astype(np.float32))
        in_maps.append({"xt": xti, "wn": wn})
    return in_maps


def kernel(x: np.ndarray, weight_: np.ndarray) -> np.ndarray:
    x = np.asarray(x)
    weight_ = np.asarray(weight_)
    res = _run(_make_in_maps(x, weight_)).results
    out_t = np.concatenate([res[i]["out_t"] for i in range(N_CORES)], axis=1)
    return np.ascontiguousarray(out_t.T).astype(np.float32)
